# revision 31
# baseline (speedup 1.0000x reference)
"""Trainium2 Bass kernel for nn_AttentionV2 (dense transformer attention block).

Reference computation (per batch element b):
    q  = Wq @ x_b  + qb          # [128, 4096]  (1x1 conv over channels)
    k  = Wk @ aux_b + kb         # [128, 4096]
    v  = Wv @ aux_b + vb         # [128, 4096]
    ktq[i, j] = sum_c k[c, i] * q[c, j]          # [4096, 4096]
    atten = softmax(ktq, axis=j)
    y[c, j] = sum_i v[c, i] * atten[i, j]        # [128, 4096]
    z = Wz @ y + zb + x_b        # [256, 4096]

Sharding: batch B=8 across the 8 cores (data parallel, weights replicated).
Each core runs the whole attention for its batch element; no collectives.

Per-core design notes:
  * All matmuls contract over the partition dim; k/q land as [c=128, hw] so
    ktq tiles need no transposes.  v is produced directly transposed
    (vT[i, c]) by using aux as the stationary operand, so the attention
    matmul y = vT.T @ exp needs no transpose either.
  * The big matmuls (ktq, y, z) run in float32r (single-pass PE, 1 cyc/row
    vs fp32's 4): every tile they consume is WRITTEN by a DVE/ACT op with a
    float32r out dtype, which rounds the mantissa as the BIR verifier
    requires.  Measured end-to-end relative error ~2e-4.
  * Softmax is computed unnormalized (no max subtraction: |ktq| <~ 30 so
    exp stays finite in fp32); the 1/rowsum factor is folded into vT
    (per-partition scalar multiply), which makes normalization free.
  * ScalarE computes exp straight out of PSUM with accum_out producing the
    row sums.  The sums land in one PERSISTENT tile (no pool rotation) so
    each exp ACTIVATE carries a single sync wait -- pool-slot reuse deps
    on a second engine would force a ~545 ns EVENT_SEMAPHORE on the
    ScalarE queue per chunk (measured; it cost ~55 us/core in v2).
  * PSUM budget (8 banks): 2 x [128, 1536] ktq chunks (6 banks) feed exp;
    2 x [128, 512] (2 banks) rotate for the grouped y accumulation and the
    final z conv.
  * i-tiles run in groups of G=4, aligned with the 512-column aux chunks:
    each group DMAs its aux chunk, computes its k columns and vT tiles
    just-in-time, then its 4 ktq/exp i-tiles, interleaved (in emission
    order) with the previous group's y matmuls so the PE keeps ScalarE fed
    across the group boundary.  q is computed for all columns up front
    (it is the ktq moving operand, needed in full by every i-tile).
"""

import sys

if "/opt/trn_rl_repo" not in sys.path:
    sys.path.insert(0, "/opt/trn_rl_repo")

import numpy as np

import concourse.bass as bass
import concourse.bacc as bacc
import concourse.mybir as mybir
import concourse.tile as tile
from concourse.masks import make_identity

DT = mybir.dt.float32
R32 = mybir.dt.float32r
P = 128          # partitions
C = 256          # input channels
CH = 128         # conv output channels (C//2)
HW = 4096        # 64*64 spatial
NJB = HW // 512  # 8 column blocks of 512
NIT = HW // P    # 32 i-tiles
G = 4            # i-tiles per group == i-tiles per 512-col aux chunk
NG = NIT // G    # 8 groups
# exp is computed in chunks straight out of PSUM; chunk layout per i-tile:
EXP_CHUNKS = ((0, 1536), (1536, 1536), (3072, 1024))

EXP_BUFS = 10
F16 = mybir.dt.float16
# softmax logits are shifted by a constant before exp so the fp16 exp tile
# cannot overflow (max logit ~26 for this distribution; softmax is
# shift-invariant and the row-sum reciprocal is computed from the same
# shifted values)
EXP_SHIFT = -17.0

Exp = mybir.ActivationFunctionType.Exp
AX = mybir.AxisListType.X


def build_module() -> bass.Bass:
    # Bacc (not plain Bass): its compile() pipeline moves extra matmul waits
    # onto LDWEIGHTS and splits >1-wait instructions (TRN2 ISA allows one
    # sync wait per instruction) -- walrus rejects the raw Tile output.
    nc = bacc.Bacc("TRN2", target_bir_lowering=False)

    x = nc.declare_dram_parameter("x", [C, HW], DT, isOutput=False)
    aux = nc.declare_dram_parameter("aux", [C, HW], DT, isOutput=False)
    # conv weights arrive pre-transposed from the host (numpy .T is free);
    # this deletes the whole PE-transpose preamble from the critical path
    WqT_d = nc.declare_dram_parameter("WqT_d", [C, CH], DT, isOutput=False)
    Wq_b = nc.declare_dram_parameter("Wq_b", [CH], DT, isOutput=False)
    WkT_d = nc.declare_dram_parameter("WkT_d", [C, CH], DT, isOutput=False)
    Wk_b = nc.declare_dram_parameter("Wk_b", [CH], DT, isOutput=False)
    WvT_d = nc.declare_dram_parameter("WvT_d", [C, CH], DT, isOutput=False)
    Wv_b = nc.declare_dram_parameter("Wv_b", [CH], DT, isOutput=False)
    WzT_d = nc.declare_dram_parameter("WzT_d", [CH, C], DT, isOutput=False)
    Wz_b = nc.declare_dram_parameter("Wz_b", [C], DT, isOutput=False)
    z = nc.declare_dram_parameter("z", [C, HW], DT, isOutput=True)

    with tile.TileContext(nc) as tc:
        with (
            tc.tile_pool(name="consts", bufs=1) as consts,
            tc.tile_pool(name="sing", bufs=1) as sing,
            tc.tile_pool(name="expp", bufs=EXP_BUFS) as expp,
            tc.tile_pool(name="instream", bufs=6) as instream,
            tc.tile_pool(name="wload", bufs=3) as wload,
            tc.tile_pool(name="smalls", bufs=6) as smalls,
            tc.tile_pool(name="xres", bufs=4) as xres,
            tc.tile_pool(name="zst", bufs=3) as zst,
            tc.tile_pool(name="psK", bufs=2, space="PSUM") as psK,
            tc.tile_pool(name="psY", bufs=2, space="PSUM") as psY,
        ):
            # DMA queue is FIFO and a dma_start whose landing slot is busy
            # BLOCKS everything behind it, so the head queue order is chosen
            # by hand: weights (gate all PE work), x cols 0-1535 (first ktq
            # chunks), aux chunk 0 (k/vT), bias rows, then the rest of x
            wts: list = []
            for w_dram in (WqT_d, WkT_d, WvT_d):
                wt = wload.tile([P, 2, P], DT, tag="wl", name="wt")
                for h in range(2):
                    nc.sync.dma_start(out=wt[:, h], in_=w_dram[h * P : (h + 1) * P, :])
                wts.append(wt)
            wtz = wload.tile([P, C], DT, tag="wlz", name="wtz", bufs=2)
            nc.sync.dma_start(out=wtz, in_=WzT_d[:, :])
            wts.append(wtz)

            xq: dict[int, tuple] = {}

            def emit_q_dma(cb: int) -> None:
                js = cb * 512
                x0 = instream.tile([P, 512], DT, tag="ins", name="x0")
                nc.sync.dma_start(out=x0, in_=x[0:P, js : js + 512])
                x1 = instream.tile([P, 512], DT, tag="ins", name="x1")
                nc.sync.dma_start(out=x1, in_=x[P:C, js : js + 512])
                xq[cb] = (x0, x1)

            for cb in range(3):
                emit_q_dma(cb)

            a0_0 = instream.tile([P, 512], DT, tag="ains", bufs=4)
            nc.sync.dma_start(out=a0_0, in_=aux[0:P, 0:512])
            a1_0 = instream.tile([P, 512], DT, tag="ains", bufs=4)
            nc.sync.dma_start(out=a1_0, in_=aux[P:C, 0:512])

            # ---- constants: biases, transposed weights ----
            ones_row = consts.tile([1, P], DT)
            nc.vector.memset(ones_row, 1.0)
            eshift = consts.tile([P, 1], DT)
            nc.vector.memset(eshift, EXP_SHIFT)
            ones512 = consts.tile([1, 512], DT)
            nc.vector.memset(ones512, 1.0)
            ones512r = consts.tile([1, 512], R32)
            nc.vector.tensor_copy(ones512r, ones512)

            # fp32r weights need a rounding producer (DVE copy); WvT (fp32
            # vT matmuls) is used straight from its DMA tile
            WkT = consts.tile([P, 2, P], R32)
            nc.vector.tensor_copy(WkT, wts[1])
            WvT = wts[2]
            WzT = consts.tile([P, 2, P], R32)
            nc.vector.tensor_copy(WzT, wts[3].rearrange("p (t q) -> p t q", t=2))

            # bias loads; qb/kb as contiguous rows (folded into the conv
            # matmuls as rank-1 accumulations -- scatter DMAs are ~2us each)
            qb_row = consts.tile([1, P], DT)
            nc.sync.dma_start(out=qb_row, in_=Wq_b[:].rearrange("(o p) -> o p", o=1))
            kb_row = consts.tile([1, P], DT)
            nc.sync.dma_start(out=kb_row, in_=Wk_b[:].rearrange("(o p) -> o p", o=1))
            kb_row_r = consts.tile([1, P], R32)
            nc.vector.tensor_copy(kb_row_r, kb_row)
            for cb in range(3, NJB):
                emit_q_dma(cb)
            zb0 = consts.tile([P, 1], DT)
            nc.sync.dma_start(out=zb0, in_=Wz_b[0:P].rearrange("(p o) -> p o", o=1))
            zb1 = consts.tile([P, 1], DT)
            nc.sync.dma_start(out=zb1, in_=Wz_b[P:C].rearrange("(p o) -> p o", o=1))
            zbias = (zb0, zb1)
            vb_row4 = consts.tile([1, G * P], DT)
            for t in range(G):
                nc.sync.dma_start(
                    out=vb_row4[:, t * P : (t + 1) * P],
                    in_=Wv_b[:].rearrange("(o p) -> o p", o=1),
                )
            # bias_bcast4[p, t*128+c] = Wv_b[c] for the batched vT bias add
            bb_ps = psK.tile([P, G * P], DT, tag="kt")
            nc.tensor.matmul(bb_ps, ones_row, vb_row4, start=True, stop=True)
            bias_bcast4 = consts.tile([P, G * P], DT)
            nc.vector.tensor_copy(bias_bcast4, bb_ps)

            # ---- persistent operands ----
            # q/k/y and the exp tiles are written by DVE/ACT with float32r
            # out dtype (rounds) so the PE can consume them single-pass.
            q_sb = sing.tile([P, HW], R32)
            k_sb = sing.tile([P, HW], R32)
            vT_sb = sing.tile([P, HW], DT)   # 32 tiles of [i=128, c=128]
            y_sb = sing.tile([P, HW], R32)
            # softmax row sums: persistent (not pooled) so the exp ACTIVATE
            # has no cross-engine slot dependency (see module docstring)
            sums = sing.tile([P, NIT, len(EXP_CHUNKS)], DT)

            # ---- q phase (interleaved with group 0 below): x chunks are
            # rounded to float32r by DVE copies so the conv matmul runs
            # single-pass ----
            def emit_q_mm(cb: int) -> None:
                js = cb * 512
                x0, x1 = xq[cb]
                qp = psK.tile([P, 512], DT, tag="kt")
                nc.tensor.matmul(qp, wts[0][:, 0], x0, start=True, stop=False)
                nc.tensor.matmul(qp, wts[0][:, 1], x1, start=False, stop=False)
                nc.tensor.matmul(qp, qb_row, ones512, start=False, stop=True)
                nc.vector.tensor_copy(q_sb[:, js : js + 512], qp)


            # ---- main loop: per group (= per aux chunk): k, vT, ktq/exp,
            #      interleaved with the previous group's y accumulation ----
            exp_t: dict[int, bass.AP] = {}
            vts_t: dict[int, bass.AP] = {}

            def emit_kv(g: int, preloaded=None) -> None:
                """DMA aux chunk g; compute k columns and vT tiles for its
                4 i-tiles."""
                js = g * 512
                if preloaded is not None:
                    a0, a1 = preloaded
                else:
                    a0 = instream.tile([P, 512], DT, tag="ains", bufs=4)
                    nc.sync.dma_start(out=a0, in_=aux[0:P, js : js + 512])
                    a1 = instream.tile([P, 512], DT, tag="ains", bufs=4)
                    nc.sync.dma_start(out=a1, in_=aux[P:C, js : js + 512])
                a0r = instream.tile([P, 512], R32, tag="ainsr", bufs=4)
                nc.vector.tensor_copy(a0r, a0)
                a1r = instream.tile([P, 512], R32, tag="ainsr", bufs=4)
                nc.vector.tensor_copy(a1r, a1)
                kp = psK.tile([P, 512], DT, tag="kt")
                nc.tensor.matmul(kp, WkT[:, 0], a0r, start=True, stop=False)
                nc.tensor.matmul(kp, WkT[:, 1], a1r, start=False, stop=False)
                nc.tensor.matmul(kp, kb_row_r, ones512r, start=False, stop=True)
                nc.vector.tensor_copy(k_sb[:, js : js + 512], kp)
                vp4 = psK.tile([P, G * P], DT, tag="kt")
                for t in range(G):
                    nc.tensor.matmul(
                        vp4[:, t * P : (t + 1) * P],
                        a0[:, t * P : (t + 1) * P], WvT[:, 0],
                        start=True, stop=False,
                    )
                    nc.tensor.matmul(
                        vp4[:, t * P : (t + 1) * P],
                        a1[:, t * P : (t + 1) * P], WvT[:, 1],
                        start=False, stop=True,
                    )
                nc.vector.tensor_add(
                    vT_sb[:, g * 512 : (g + 1) * 512], vp4, bias_bcast4
                )

            def emit_a_chunk(it: int, ci: int) -> None:
                """ktq + exp for one (i-tile, column chunk)."""
                if ci == 0:
                    exp_t[it] = expp.tile([P, HW], F16, tag="exp", name="et")
                et = exp_t[it]
                off, w = EXP_CHUNKS[ci]
                kt = psK.tile([P, w], DT, tag="kt")
                for s in range(w // 512):
                    nc.tensor.matmul(
                        kt[:, s * 512 : (s + 1) * 512],
                        k_sb[:, it * P : (it + 1) * P],
                        q_sb[:, off + s * 512 : off + (s + 1) * 512],
                        start=True, stop=True,
                    )
                nc.scalar.activation(
                    out=et[:, off : off + w], in_=kt, func=Exp,
                    bias=eshift, scale=1.0,
                    accum_out=sums[:, it, ci : ci + 1],
                )

            def emit_a_fin(it: int) -> None:
                """softmax row-sum reciprocal folded into vT."""
                sv = smalls.tile([P, 1], DT, tag="sv")
                nc.vector.reduce_sum(sv, sums[:, it], axis=AX)
                rv = smalls.tile([P, 1], DT, tag="rv")
                nc.vector.reciprocal(rv, sv)
                vt = smalls.tile([P, P], F16, tag="vts")
                nc.vector.tensor_scalar_mul(vt, vT_sb[:, it * P : (it + 1) * P], rv)
                vts_t[it] = vt

            def emit_a(it: int) -> None:
                for ci in range(len(EXP_CHUNKS)):
                    emit_a_chunk(it, ci)
                emit_a_fin(it)

            def emit_b(g: int, jb: int) -> None:
                """y[:, jb] += vts.T @ exp for the 4 i-tiles of group g."""
                js = jb * 512
                yp = psY.tile([P, 512], DT, tag="y")
                grp = range(g * G, (g + 1) * G)
                for gi, it in enumerate(grp):
                    nc.tensor.matmul(
                        yp, vts_t[it], exp_t[it][:, js : js + 512],
                        start=(gi == 0), stop=(gi == G - 1),
                    )
                if g == 0:
                    nc.vector.tensor_copy(y_sb[:, js : js + 512], yp)
                else:
                    nc.vector.tensor_add(
                        y_sb[:, js : js + 512], y_sb[:, js : js + 512], yp
                    )

            # group 0 interleaves with the q chunks it needs (exp chunk
            # boundaries 0/1536/3072 line up with q chunks 0-2, 3-5, 6-7);
            # group g+1's k/vT are emitted mid-group-g so the PE has them
            # ready before ScalarE finishes group g's exps.
            # group 0 is emitted chunk-wise BEHIND the q columns each chunk
            # reads (Tile deps are program-order: a ktq matmul must be
            # emitted after the q writes it consumes)
            for cb in range(3):
                emit_q_mm(cb)
            emit_kv(0, preloaded=(a0_0, a1_0))
            for t in range(G):
                emit_a_chunk(t, 0)
            for cb in range(3, 6):
                emit_q_mm(cb)
            for t in range(G):
                emit_a_chunk(t, 1)
            for cb in range(6, 8):
                emit_q_mm(cb)
            emit_kv(1)
            for t in range(G):
                emit_a_chunk(t, 2)
                emit_a_fin(t)
            for g in range(1, NG):
                # weave the previous group's y-blocks BETWEEN ktq chunks:
                # the PE runs one chunk ahead of ScalarE (2 psum slots), so
                # a 4-matmul y-block is exactly the filler for the ~1us the
                # PE would otherwise stall waiting for exp to drain a slot
                jb_cursor = 0
                nb_per_t = (3, 3, 2, 0)
                for t in range(G):
                    it = g * G + t
                    for ci in range(len(EXP_CHUNKS)):
                        emit_a_chunk(it, ci)
                        if ci < nb_per_t[t] and jb_cursor < NJB:
                            emit_b(g - 1, jb_cursor)
                            jb_cursor += 1
                    emit_a_fin(it)
                    if t == 1 and g + 1 < NG:
                        emit_kv(g + 1)

            # ---- tail: last group's y + z = Wz @ y + zb + x, streamed per
            #      column block as soon as its final y flush lands ----
            Identity = mybir.ActivationFunctionType.Identity
            xr: dict[tuple[int, int], bass.AP] = {}

            def emit_xres(jb: int) -> None:
                js = jb * 512
                for h in range(2):
                    xt = xres.tile([P, 512], DT, tag="xr")
                    nc.sync.dma_start(out=xt, in_=x[h * P : (h + 1) * P, js : js + 512])
                    xr[(jb, h)] = xt

            def emit_z(jb: int) -> None:
                js = jb * 512
                for h in range(2):
                    # psK slots are free once the ktq stream ends; using them
                    # gives the tail a 4-deep psum pipeline instead of
                    # fighting the y flushes for the 2 psY slots
                    zp = psK.tile([P, 512], DT, tag="kt")
                    nc.tensor.matmul(
                        zp, WzT[:, h], y_sb[:, js : js + 512], start=True, stop=True
                    )
                    zc = zst.tile([P, 512], DT, tag="zc")
                    nc.scalar.activation(out=zc, in_=zp, func=Identity, bias=zbias[h])
                    nc.vector.tensor_add(zc, zc, xr[(jb, h)])
                    nc.sync.dma_start(out=z[h * P : (h + 1) * P, js : js + 512], in_=zc)

            emit_xres(0)
            emit_b(NG - 1, 0)
            for jb in range(1, NJB):
                emit_xres(jb)
                emit_b(NG - 1, jb)
                emit_z(jb - 1)
            emit_z(NJB - 1)

    nc.compile()
    return nc


_NC = None


def _get_nc() -> bass.Bass:
    global _NC
    if _NC is None:
        _NC = build_module()
    return _NC


def _make_in_maps(inputs: dict[str, np.ndarray]) -> list[dict[str, np.ndarray]]:
    B = inputs["x"].shape[0]
    shared = {
        name: np.ascontiguousarray(np.asarray(inputs[name], dtype=np.float32))
        for name in ("Wq_b", "Wk_b", "Wv_b", "Wz_b")
    }
    for dev_name, host_name in (
        ("WqT_d", "Wq_w"), ("WkT_d", "Wk_w"), ("WvT_d", "Wv_w"), ("WzT_d", "Wz_w"),
    ):
        shared[dev_name] = np.ascontiguousarray(
            np.asarray(inputs[host_name], dtype=np.float32).T
        )
    in_maps = []
    for b in range(B):
        m = dict(shared)
        m["x"] = np.ascontiguousarray(
            np.asarray(inputs["x"][b], dtype=np.float32).reshape(C, HW)
        )
        m["aux"] = np.ascontiguousarray(
            np.asarray(inputs["aux"][b], dtype=np.float32).reshape(C, HW)
        )
        in_maps.append(m)
    return in_maps


def _install_ntff_hook_shim() -> None:
    """The agent image's antenv lacks axon_hooks; recreate it so
    run_bass_kernel_spmd(trace=True) can reach the libaxon NTFF profiler."""
    import types

    if "antenv.axon_hooks" in sys.modules:
        return
    import antenv

    mod = types.ModuleType("antenv.axon_hooks")
    state = {"hook": None}
    mod.set_axon_ntff_profile_hook = lambda h: state.__setitem__("hook", h)
    mod.get_axon_ntff_profile_hook = lambda: state["hook"]
    sys.modules["antenv.axon_hooks"] = mod
    antenv.axon_hooks = mod
    try:
        from trn_agent_boot.trn_boot import _ntff_profile_via_ctypes

        hook = _ntff_profile_via_ctypes("/opt/axon/libaxon_pjrt.so")
        if hook is not None:
            mod.set_axon_ntff_profile_hook(hook)
    except Exception as e:  # degrade to no tracing
        print(f"ntff hook unavailable: {e}", file=sys.stderr)


def run(inputs: dict[str, np.ndarray], trace: bool = False):
    """Run on the 8 NeuronCores; returns (output [8,256,64,64], BassKernelResults)."""
    from concourse.bass_utils import run_bass_kernel_spmd

    if trace:
        _install_ntff_hook_shim()
    nc = _get_nc()
    in_maps = _make_in_maps(inputs)
    res = run_bass_kernel_spmd(nc, in_maps, list(range(len(in_maps))), trace=trace)
    out = np.stack([r["z"].reshape(C, 64, 64) for r in res.results])
    return out.astype(np.float32), res


def kernel(**inputs: np.ndarray) -> np.ndarray:
    out, _ = run(inputs, trace=False)
    return out


if __name__ == "__main__":
    nc = build_module()
    print("module built ok")


# revision 32
# speedup vs baseline: 1.0049x; 1.0049x over previous
"""Trainium2 Bass kernel for nn_AttentionV2 (dense transformer attention block).

Reference computation (per batch element b):
    q  = Wq @ x_b  + qb          # [128, 4096]  (1x1 conv over channels)
    k  = Wk @ aux_b + kb         # [128, 4096]
    v  = Wv @ aux_b + vb         # [128, 4096]
    ktq[i, j] = sum_c k[c, i] * q[c, j]          # [4096, 4096]
    atten = softmax(ktq, axis=j)
    y[c, j] = sum_i v[c, i] * atten[i, j]        # [128, 4096]
    z = Wz @ y + zb + x_b        # [256, 4096]

Sharding: batch B=8 across the 8 cores (data parallel, weights replicated).
Each core runs the whole attention for its batch element; no collectives.

Per-core design notes:
  * All matmuls contract over the partition dim; k/q land as [c=128, hw] so
    ktq tiles need no transposes.  v is produced directly transposed
    (vT[i, c]) by using aux as the stationary operand, so the attention
    matmul y = vT.T @ exp needs no transpose either.
  * The big matmuls (ktq, y, z) run in float32r (single-pass PE, 1 cyc/row
    vs fp32's 4): every tile they consume is WRITTEN by a DVE/ACT op with a
    float32r out dtype, which rounds the mantissa as the BIR verifier
    requires.  Measured end-to-end relative error ~2e-4.
  * Softmax is computed unnormalized (no max subtraction: |ktq| <~ 30 so
    exp stays finite in fp32); the 1/rowsum factor is folded into vT
    (per-partition scalar multiply), which makes normalization free.
  * ScalarE computes exp straight out of PSUM with accum_out producing the
    row sums.  The sums land in one PERSISTENT tile (no pool rotation) so
    each exp ACTIVATE carries a single sync wait -- pool-slot reuse deps
    on a second engine would force a ~545 ns EVENT_SEMAPHORE on the
    ScalarE queue per chunk (measured; it cost ~55 us/core in v2).
  * PSUM budget (8 banks): 2 x [128, 1536] ktq chunks (6 banks) feed exp;
    2 x [128, 512] (2 banks) rotate for the grouped y accumulation and the
    final z conv.
  * i-tiles run in groups of G=4, aligned with the 512-column aux chunks:
    each group DMAs its aux chunk, computes its k columns and vT tiles
    just-in-time, then its 4 ktq/exp i-tiles, interleaved (in emission
    order) with the previous group's y matmuls so the PE keeps ScalarE fed
    across the group boundary.  q is computed for all columns up front
    (it is the ktq moving operand, needed in full by every i-tile).
"""

import sys

if "/opt/trn_rl_repo" not in sys.path:
    sys.path.insert(0, "/opt/trn_rl_repo")

import numpy as np

import concourse.bass as bass
import concourse.bacc as bacc
import concourse.mybir as mybir
import concourse.tile as tile
from concourse.masks import make_identity

DT = mybir.dt.float32
R32 = mybir.dt.float32r
P = 128          # partitions
C = 256          # input channels
CH = 128         # conv output channels (C//2)
HW = 4096        # 64*64 spatial
NJB = HW // 512  # 8 column blocks of 512
NIT = HW // P    # 32 i-tiles
G = 4            # i-tiles per group == i-tiles per 512-col aux chunk
NG = NIT // G    # 8 groups
# exp is computed in chunks straight out of PSUM; chunk layout per i-tile:
EXP_CHUNKS = ((0, 1536), (1536, 1536), (3072, 1024))

EXP_BUFS = 10
F16 = mybir.dt.float16
# softmax logits are shifted by a constant before exp so the fp16 exp tile
# cannot overflow (max logit ~26 for this distribution; softmax is
# shift-invariant and the row-sum reciprocal is computed from the same
# shifted values)
EXP_SHIFT = -17.0

Exp = mybir.ActivationFunctionType.Exp
AX = mybir.AxisListType.X


def build_module() -> bass.Bass:
    # Bacc (not plain Bass): its compile() pipeline moves extra matmul waits
    # onto LDWEIGHTS and splits >1-wait instructions (TRN2 ISA allows one
    # sync wait per instruction) -- walrus rejects the raw Tile output.
    nc = bacc.Bacc("TRN2", target_bir_lowering=False)

    x = nc.declare_dram_parameter("x", [C, HW], DT, isOutput=False)
    aux = nc.declare_dram_parameter("aux", [C, HW], DT, isOutput=False)
    # conv weights arrive pre-transposed from the host (numpy .T is free);
    # this deletes the whole PE-transpose preamble from the critical path
    WqT_d = nc.declare_dram_parameter("WqT_d", [C, CH], DT, isOutput=False)
    Wq_b = nc.declare_dram_parameter("Wq_b", [CH], DT, isOutput=False)
    WkT_d = nc.declare_dram_parameter("WkT_d", [C, CH], DT, isOutput=False)
    Wk_b = nc.declare_dram_parameter("Wk_b", [CH], DT, isOutput=False)
    WvT_d = nc.declare_dram_parameter("WvT_d", [C, CH], DT, isOutput=False)
    Wv_b = nc.declare_dram_parameter("Wv_b", [CH], DT, isOutput=False)
    WzT_d = nc.declare_dram_parameter("WzT_d", [CH, C], DT, isOutput=False)
    Wz_b = nc.declare_dram_parameter("Wz_b", [C], DT, isOutput=False)
    z = nc.declare_dram_parameter("z", [C, HW], DT, isOutput=True)

    with tile.TileContext(nc) as tc:
        with (
            tc.tile_pool(name="consts", bufs=1) as consts,
            tc.tile_pool(name="sing", bufs=1) as sing,
            tc.tile_pool(name="expp", bufs=EXP_BUFS) as expp,
            tc.tile_pool(name="instream", bufs=6) as instream,
            tc.tile_pool(name="wload", bufs=3) as wload,
            tc.tile_pool(name="smalls", bufs=6) as smalls,
            tc.tile_pool(name="xres", bufs=4) as xres,
            tc.tile_pool(name="zst", bufs=3) as zst,
            tc.tile_pool(name="psK", bufs=2, space="PSUM") as psK,
            tc.tile_pool(name="psY", bufs=2, space="PSUM") as psY,
        ):
            # DMA queue is FIFO and a dma_start whose landing slot is busy
            # BLOCKS everything behind it, so the head queue order is chosen
            # by hand: weights (gate all PE work), x cols 0-1535 (first ktq
            # chunks), aux chunk 0 (k/vT), bias rows, then the rest of x
            wts: list = []
            for w_dram in (WqT_d, WkT_d, WvT_d):
                wt = wload.tile([P, 2, P], DT, tag="wl", name="wt")
                for h in range(2):
                    nc.sync.dma_start(out=wt[:, h], in_=w_dram[h * P : (h + 1) * P, :])
                wts.append(wt)
            wtz = wload.tile([P, C], DT, tag="wlz", name="wtz", bufs=2)
            nc.sync.dma_start(out=wtz, in_=WzT_d[:, :])
            wts.append(wtz)

            xq: dict[int, tuple] = {}

            def emit_q_dma(cb: int) -> None:
                js = cb * 512
                x0 = instream.tile([P, 512], DT, tag="ins", name="x0")
                nc.sync.dma_start(out=x0, in_=x[0:P, js : js + 512])
                x1 = instream.tile([P, 512], DT, tag="ins", name="x1")
                nc.sync.dma_start(out=x1, in_=x[P:C, js : js + 512])
                xq[cb] = (x0, x1)

            for cb in range(3):
                emit_q_dma(cb)

            a0_0 = instream.tile([P, 512], DT, tag="ains", bufs=4)
            nc.sync.dma_start(out=a0_0, in_=aux[0:P, 0:512])
            a1_0 = instream.tile([P, 512], DT, tag="ains", bufs=4)
            nc.sync.dma_start(out=a1_0, in_=aux[P:C, 0:512])

            # ---- constants: biases, transposed weights ----
            ones_row = consts.tile([1, P], DT)
            nc.vector.memset(ones_row, 1.0)
            eshift = consts.tile([P, 1], DT)
            nc.vector.memset(eshift, EXP_SHIFT)
            ones512 = consts.tile([1, 512], DT)
            nc.vector.memset(ones512, 1.0)
            ones512r = consts.tile([1, 512], R32)
            nc.vector.tensor_copy(ones512r, ones512)

            # fp32r weights need a rounding producer (DVE copy); WvT (fp32
            # vT matmuls) is used straight from its DMA tile
            WkT = consts.tile([P, 2, P], R32)
            nc.vector.tensor_copy(WkT, wts[1])
            WvT = wts[2]
            WzT = consts.tile([P, 2, P], R32)
            nc.vector.tensor_copy(WzT, wts[3].rearrange("p (t q) -> p t q", t=2))

            # bias loads; qb/kb as contiguous rows (folded into the conv
            # matmuls as rank-1 accumulations -- scatter DMAs are ~2us each)
            qb_row = consts.tile([1, P], DT)
            nc.sync.dma_start(out=qb_row, in_=Wq_b[:].rearrange("(o p) -> o p", o=1))
            kb_row = consts.tile([1, P], DT)
            nc.sync.dma_start(out=kb_row, in_=Wk_b[:].rearrange("(o p) -> o p", o=1))
            kb_row_r = consts.tile([1, P], R32)
            nc.vector.tensor_copy(kb_row_r, kb_row)
            for cb in range(3, NJB):
                emit_q_dma(cb)
            zb0 = consts.tile([P, 1], DT)
            nc.sync.dma_start(out=zb0, in_=Wz_b[0:P].rearrange("(p o) -> p o", o=1))
            zb1 = consts.tile([P, 1], DT)
            nc.sync.dma_start(out=zb1, in_=Wz_b[P:C].rearrange("(p o) -> p o", o=1))
            zbias = (zb0, zb1)
            vb_row4 = consts.tile([1, G * P], DT)
            for t in range(G):
                nc.sync.dma_start(
                    out=vb_row4[:, t * P : (t + 1) * P],
                    in_=Wv_b[:].rearrange("(o p) -> o p", o=1),
                )
            # bias_bcast4[p, t*128+c] = Wv_b[c] for the batched vT bias add
            bb_ps = psK.tile([P, G * P], DT, tag="kt")
            nc.tensor.matmul(bb_ps, ones_row, vb_row4, start=True, stop=True)
            bias_bcast4 = consts.tile([P, G * P], DT)
            nc.vector.tensor_copy(bias_bcast4, bb_ps)

            # ---- persistent operands ----
            # q/k/y and the exp tiles are written by DVE/ACT with float32r
            # out dtype (rounds) so the PE can consume them single-pass.
            q_sb = sing.tile([P, HW], R32)
            k_sb = sing.tile([P, HW], R32)
            vT_sb = sing.tile([P, HW], DT)   # 32 tiles of [i=128, c=128]
            y_sb = sing.tile([P, HW], R32)
            # softmax row sums: persistent (not pooled) so the exp ACTIVATE
            # has no cross-engine slot dependency (see module docstring)
            sums = sing.tile([P, NIT, len(EXP_CHUNKS)], DT)

            # ---- q phase (interleaved with group 0 below): x chunks are
            # rounded to float32r by DVE copies so the conv matmul runs
            # single-pass ----
            def emit_q_mm(cb: int) -> None:
                js = cb * 512
                x0, x1 = xq[cb]
                qp = psK.tile([P, 512], DT, tag="kt")
                nc.tensor.matmul(qp, wts[0][:, 0], x0, start=True, stop=False)
                nc.tensor.matmul(qp, wts[0][:, 1], x1, start=False, stop=False)
                nc.tensor.matmul(qp, qb_row, ones512, start=False, stop=True)
                nc.vector.tensor_copy(q_sb[:, js : js + 512], qp)


            # ---- main loop: per group (= per aux chunk): k, vT, ktq/exp,
            #      interleaved with the previous group's y accumulation ----
            exp_t: dict[int, bass.AP] = {}
            vts_t: dict[int, bass.AP] = {}

            kvt: dict[int, tuple] = {}

            def emit_kv_dma(g: int, preloaded=None) -> None:
                js = g * 512
                if preloaded is not None:
                    a0, a1 = preloaded
                else:
                    a0 = instream.tile([P, 512], DT, tag="ains", bufs=4)
                    nc.sync.dma_start(out=a0, in_=aux[0:P, js : js + 512])
                    a1 = instream.tile([P, 512], DT, tag="ains", bufs=4)
                    nc.sync.dma_start(out=a1, in_=aux[P:C, js : js + 512])
                a0r = instream.tile([P, 512], R32, tag="ainsr", bufs=4)
                nc.vector.tensor_copy(a0r, a0)
                a1r = instream.tile([P, 512], R32, tag="ainsr", bufs=4)
                nc.vector.tensor_copy(a1r, a1)
                kvt[g] = (a0, a1, a0r, a1r)

            def emit_kv_k(g: int) -> None:
                js = g * 512
                a0, a1, a0r, a1r = kvt[g]
                kp = psK.tile([P, 512], DT, tag="kt")
                nc.tensor.matmul(kp, WkT[:, 0], a0r, start=True, stop=False)
                nc.tensor.matmul(kp, WkT[:, 1], a1r, start=False, stop=False)
                nc.tensor.matmul(kp, kb_row_r, ones512r, start=False, stop=True)
                nc.vector.tensor_copy(k_sb[:, js : js + 512], kp)

            def emit_kv_v(g: int, half: int) -> None:
                a0, a1, a0r, a1r = kvt[g]
                vp2 = psK.tile([P, 2 * P], DT, tag="kt")
                for ti in range(2):
                    t = half * 2 + ti
                    nc.tensor.matmul(
                        vp2[:, ti * P : (ti + 1) * P],
                        a0[:, t * P : (t + 1) * P], WvT[:, 0],
                        start=True, stop=False,
                    )
                    nc.tensor.matmul(
                        vp2[:, ti * P : (ti + 1) * P],
                        a1[:, t * P : (t + 1) * P], WvT[:, 1],
                        start=False, stop=True,
                    )
                off = g * 512 + half * 256
                nc.vector.tensor_add(
                    vT_sb[:, off : off + 256], vp2, bias_bcast4[:, 0:256]
                )

            def emit_kv(g: int, preloaded=None) -> None:
                emit_kv_dma(g, preloaded)
                emit_kv_k(g)
                emit_kv_v(g, 0)
                emit_kv_v(g, 1)

            def emit_a_chunk(it: int, ci: int) -> None:
                """ktq + exp for one (i-tile, column chunk)."""
                if ci == 0:
                    exp_t[it] = expp.tile([P, HW], F16, tag="exp", name="et")
                et = exp_t[it]
                off, w = EXP_CHUNKS[ci]
                kt = psK.tile([P, w], DT, tag="kt")
                for s in range(w // 512):
                    nc.tensor.matmul(
                        kt[:, s * 512 : (s + 1) * 512],
                        k_sb[:, it * P : (it + 1) * P],
                        q_sb[:, off + s * 512 : off + (s + 1) * 512],
                        start=True, stop=True,
                    )
                nc.scalar.activation(
                    out=et[:, off : off + w], in_=kt, func=Exp,
                    bias=eshift, scale=1.0,
                    accum_out=sums[:, it, ci : ci + 1],
                )

            def emit_a_fin(it: int) -> None:
                """softmax row-sum reciprocal folded into vT."""
                sv = smalls.tile([P, 1], DT, tag="sv")
                nc.vector.reduce_sum(sv, sums[:, it], axis=AX)
                rv = smalls.tile([P, 1], DT, tag="rv")
                nc.vector.reciprocal(rv, sv)
                vt = smalls.tile([P, P], F16, tag="vts")
                nc.vector.tensor_scalar_mul(vt, vT_sb[:, it * P : (it + 1) * P], rv)
                vts_t[it] = vt

            def emit_a(it: int) -> None:
                for ci in range(len(EXP_CHUNKS)):
                    emit_a_chunk(it, ci)
                emit_a_fin(it)

            def emit_b(g: int, jb: int) -> None:
                """y[:, jb] += vts.T @ exp for the 4 i-tiles of group g."""
                js = jb * 512
                yp = psY.tile([P, 512], DT, tag="y")
                grp = range(g * G, (g + 1) * G)
                for gi, it in enumerate(grp):
                    nc.tensor.matmul(
                        yp, vts_t[it], exp_t[it][:, js : js + 512],
                        start=(gi == 0), stop=(gi == G - 1),
                    )
                if g == 0:
                    nc.vector.tensor_copy(y_sb[:, js : js + 512], yp)
                else:
                    nc.vector.tensor_add(
                        y_sb[:, js : js + 512], y_sb[:, js : js + 512], yp
                    )

            # group 0 interleaves with the q chunks it needs (exp chunk
            # boundaries 0/1536/3072 line up with q chunks 0-2, 3-5, 6-7);
            # group g+1's k/vT are emitted mid-group-g so the PE has them
            # ready before ScalarE finishes group g's exps.
            # group 0 is emitted chunk-wise BEHIND the q columns each chunk
            # reads (Tile deps are program-order: a ktq matmul must be
            # emitted after the q writes it consumes)
            for cb in range(3):
                emit_q_mm(cb)
            emit_kv(0, preloaded=(a0_0, a1_0))
            for t in range(G):
                emit_a_chunk(t, 0)
            for cb in range(3, 6):
                emit_q_mm(cb)
            for t in range(G):
                emit_a_chunk(t, 1)
            for cb in range(6, 8):
                emit_q_mm(cb)
            emit_kv(1)
            for t in range(G):
                emit_a_chunk(t, 2)
                emit_a_fin(t)
            for g in range(1, NG):
                # weave the previous group's y-blocks AND the next group's
                # k/vT pieces BETWEEN ktq chunks: a contiguous kv block at
                # the boundary gives ScalarE a measured ~3.6us hole
                jb_cursor = 0
                nb_per_t = (3, 3, 2, 0)
                for t in range(G):
                    it = g * G + t
                    for ci in range(len(EXP_CHUNKS)):
                        emit_a_chunk(it, ci)
                        if ci < nb_per_t[t] and jb_cursor < NJB:
                            emit_b(g - 1, jb_cursor)
                            jb_cursor += 1
                        if g + 1 < NG:
                            if t == 1 and ci == 0:
                                emit_kv_dma(g + 1)
                            elif t == 1 and ci == 1:
                                emit_kv_k(g + 1)
                            elif t == 1 and ci == 2:
                                emit_kv_v(g + 1, 0)
                            elif t == 2 and ci == 0:
                                emit_kv_v(g + 1, 1)
                    emit_a_fin(it)
            # ---- tail            # ---- tail: last group's y + z = Wz @ y + zb + x, streamed per
            #      column block as soon as its final y flush lands ----
            Identity = mybir.ActivationFunctionType.Identity
            xr: dict[tuple[int, int], bass.AP] = {}

            def emit_xres(jb: int) -> None:
                js = jb * 512
                for h in range(2):
                    xt = xres.tile([P, 512], DT, tag="xr")
                    nc.sync.dma_start(out=xt, in_=x[h * P : (h + 1) * P, js : js + 512])
                    xr[(jb, h)] = xt

            def emit_z(jb: int) -> None:
                js = jb * 512
                for h in range(2):
                    # psK slots are free once the ktq stream ends; using them
                    # gives the tail a 4-deep psum pipeline instead of
                    # fighting the y flushes for the 2 psY slots
                    zp = psK.tile([P, 512], DT, tag="kt")
                    nc.tensor.matmul(
                        zp, WzT[:, h], y_sb[:, js : js + 512], start=True, stop=True
                    )
                    zc = zst.tile([P, 512], DT, tag="zc")
                    nc.scalar.activation(out=zc, in_=zp, func=Identity, bias=zbias[h])
                    nc.vector.tensor_add(zc, zc, xr[(jb, h)])
                    nc.sync.dma_start(out=z[h * P : (h + 1) * P, js : js + 512], in_=zc)

            emit_xres(0)
            emit_b(NG - 1, 0)
            for jb in range(1, NJB):
                emit_xres(jb)
                emit_b(NG - 1, jb)
                emit_z(jb - 1)
            emit_z(NJB - 1)

    nc.compile()
    return nc


_NC = None


def _get_nc() -> bass.Bass:
    global _NC
    if _NC is None:
        _NC = build_module()
    return _NC


def _make_in_maps(inputs: dict[str, np.ndarray]) -> list[dict[str, np.ndarray]]:
    B = inputs["x"].shape[0]
    shared = {
        name: np.ascontiguousarray(np.asarray(inputs[name], dtype=np.float32))
        for name in ("Wq_b", "Wk_b", "Wv_b", "Wz_b")
    }
    for dev_name, host_name in (
        ("WqT_d", "Wq_w"), ("WkT_d", "Wk_w"), ("WvT_d", "Wv_w"), ("WzT_d", "Wz_w"),
    ):
        shared[dev_name] = np.ascontiguousarray(
            np.asarray(inputs[host_name], dtype=np.float32).T
        )
    in_maps = []
    for b in range(B):
        m = dict(shared)
        m["x"] = np.ascontiguousarray(
            np.asarray(inputs["x"][b], dtype=np.float32).reshape(C, HW)
        )
        m["aux"] = np.ascontiguousarray(
            np.asarray(inputs["aux"][b], dtype=np.float32).reshape(C, HW)
        )
        in_maps.append(m)
    return in_maps


def _install_ntff_hook_shim() -> None:
    """The agent image's antenv lacks axon_hooks; recreate it so
    run_bass_kernel_spmd(trace=True) can reach the libaxon NTFF profiler."""
    import types

    if "antenv.axon_hooks" in sys.modules:
        return
    import antenv

    mod = types.ModuleType("antenv.axon_hooks")
    state = {"hook": None}
    mod.set_axon_ntff_profile_hook = lambda h: state.__setitem__("hook", h)
    mod.get_axon_ntff_profile_hook = lambda: state["hook"]
    sys.modules["antenv.axon_hooks"] = mod
    antenv.axon_hooks = mod
    try:
        from trn_agent_boot.trn_boot import _ntff_profile_via_ctypes

        hook = _ntff_profile_via_ctypes("/opt/axon/libaxon_pjrt.so")
        if hook is not None:
            mod.set_axon_ntff_profile_hook(hook)
    except Exception as e:  # degrade to no tracing
        print(f"ntff hook unavailable: {e}", file=sys.stderr)


def run(inputs: dict[str, np.ndarray], trace: bool = False):
    """Run on the 8 NeuronCores; returns (output [8,256,64,64], BassKernelResults)."""
    from concourse.bass_utils import run_bass_kernel_spmd

    if trace:
        _install_ntff_hook_shim()
    nc = _get_nc()
    in_maps = _make_in_maps(inputs)
    res = run_bass_kernel_spmd(nc, in_maps, list(range(len(in_maps))), trace=trace)
    out = np.stack([r["z"].reshape(C, 64, 64) for r in res.results])
    return out.astype(np.float32), res


def kernel(**inputs: np.ndarray) -> np.ndarray:
    out, _ = run(inputs, trace=False)
    return out


if __name__ == "__main__":
    nc = build_module()
    print("module built ok")
